# revision 29
# baseline (speedup 1.0000x reference)
"""GAT multi-head attention kernel for Trainium2 (8 NeuronCores, data-parallel over batch).

Problem (hardcoded): h [8,1024,128] f32, W [8,128,16] f32, Wa [8,32] f32.
  g   = einsum('bni,hid->hbnd', h, W)
  e   = leakyrelu(g@a_src [i] + g@a_dst [j], 0.2)      # [H,B,N,N]
  P   = softmax_j(e)
  out = relu(einsum('hbij,hbjd->bihd', P, g)).reshape(B,N,H*hd)

Sharding: graph b -> core b. Each core computes all 8 heads for its graph.

Algebra: with s=g@a_src (per-i), t=g@a_dst (per-j),
  exp(leakyrelu(s_i+t_j)) = max(e^{s_i+t_j}, e^{.2(s_i+t_j)})
                          = e^{.2 s_i} * e^{t_j} * max(e^{.8 s_i}, e^{-.8 t_j})
The e^{.2 s_i} factor is constant along the softmax axis j, so it cancels in
the softmax ratio and is simply dropped. The e^{t_j} factor rides along the
contraction dim of the attention matmul and is folded into the stationary
[g|1] (whose ones column also yields the softmax denominator). Each NxN
probability tile costs ONE fused dual-op vector instruction:
  u'_ij = max(q_i, r_j) * F1_j,  q = e^{.8 s} (rows, DMA-broadcast across
  partitions), r = e^{-.8 t} and F1 = e^t (per-partition columns),
via tensor_scalar(op0=max, op1=mult) on the vector engine (bf16, 2x mode).
All u tiles run on DVE; the attention matmuls (bf16, 4 heads packed per PSUM
accumulator at partition offsets 32*hh via col-group tile_position) pace the
kernel. The q rows come from the same s/t column matmul as the per-partition
factors (via 8 small PE transposes + partition-broadcast DMAs spread across
the SP/GPSIMD/ACT DGE queues). PSUM->SBUF copies fold the output relu; phase
F does one bf16 [128,128] PE transpose per (group, i-chunk), a strided
reciprocal, and a single stride-0-broadcast tensor_tensor multiply for the
softmax normalization, then streams the finished i-chunk straight to HBM.
"""
import numpy as np
import ml_dtypes
from contextlib import ExitStack

import concourse.bass as bass
import concourse.tile as tile
from concourse import bacc, mybir
from concourse import bass_utils

# ---- problem constants (from spec; kernel.py must be self-contained) ----
B, N, DI, H, HD = 8, 1024, 128, 8, 16
SLOPE = 0.2
NC128 = N // 128            # 8 chunks of 128
FP32 = mybir.dt.float32
F32R = mybir.dt.float32r
BF16 = mybir.dt.bfloat16

AF = mybir.ActivationFunctionType
ALU = mybir.AluOpType

DT_MM = BF16          # dtype of probability tiles + attention-matmul operands
QB_VIA_DRAM = True    # bounce q rows through DRAM (SBUF/PSUM can't stride-0)


def build_nc(iters: int = 1, variant: str = "full"):
    nc = bacc.Bacc("TRN2", target_bir_lowering=False, debug=False, num_devices=8)

    hb_d = nc.dram_tensor("hb", [N, DI], F32R, kind="ExternalInput")
    wall_d = nc.dram_tensor("wall", [DI, H * HD], BF16, kind="ExternalInput")
    wabd_d = nc.dram_tensor("wabd", [DI, 2 * H], BF16, kind="ExternalInput")
    ident_d = nc.dram_tensor("ident", [128, 128], F32R, kind="ExternalInput")
    identb_d = nc.dram_tensor("identb", [128, 128], BF16, kind="ExternalInput")
    out_d = nc.dram_tensor("out", [N, H * HD], FP32, kind="ExternalOutput")

    with tile.TileContext(nc) as tc:
        with ExitStack() as ctx:
            if iters > 1:
                ctx.enter_context(tc.For_i(
                    0, iters, 1,
                    hint_engines=(mybir.EngineType.PE, mybir.EngineType.DVE,
                                  mybir.EngineType.Activation,
                                  mybir.EngineType.SP)))
            _body(ctx, tc, hb_d, wall_d, wabd_d, ident_d, identb_d, out_d,
                  variant)
    nc.compile()
    return nc


def _bcast16(ap):
    """[128, A]-style AP -> [128, A, 16] via an appended stride-0 free dim."""
    new = ap.copy()
    new.ap = ap.ap + [[0, 16]]
    return new


def _emit_mm(nc, o4, g_ext, mts, h, hh, jcs, spans):
    for jc in jcs:
        for lo, hi in spans:
            lhsT = g_ext[:, jc * 256 + h * 32: jc * 256 + (h + 1) * 32]
            nc.tensor.matmul(
                o4[32 * hh:32 * hh + 32, lo:hi],
                lhsT, mts[jc][:, lo:hi],
                start=(jc == 0), stop=(jc == NC128 - 1),
                tile_position=(0, 32 * hh), skip_group_check=True)


def _body(ctx, tc, hb_d, wall_d, wabd_d, ident_d, identb_d, out_d,
          variant="full"):
    nc = tc.nc
    consts = ctx.enter_context(tc.tile_pool(name="consts", bufs=1))
    sb = ctx.enter_context(tc.tile_pool(name="sb", bufs=4))
    mtp = ctx.enter_context(tc.tile_pool(name="mtp", bufs=20))
    ps_small = ctx.enter_context(tc.tile_pool(name="ps_small", bufs=4, space="PSUM"))
    ps_oh = ctx.enter_context(tc.tile_pool(name="ps_oh", bufs=2, space="PSUM"))
    dram = ctx.enter_context(tc.tile_pool(name="dram", bufs=1, space="DRAM"))

    # ---- constants in: small consts on the GPSIMD DGE, h on SP + ACT DGEs ----
    ident = consts.tile([128, 128], F32R)
    nc.gpsimd.dma_start(ident[:], ident_d.ap())
    identb = consts.tile([128, 128], BF16)
    nc.gpsimd.dma_start(identb[:], identb_d.ap())
    wall = consts.tile([128, H * HD], BF16)
    nc.gpsimd.dma_start(wall[:], wall_d.ap())
    wq = consts.tile([128, 2 * H], BF16)
    nc.gpsimd.dma_start(wq[:], wabd_d.ap())

    hall = consts.tile([128, N], F32R)   # [p, c*128+i] = hb[c*128+p, i]
    for half, eng in ((0, nc.sync), (1, nc.sync)):
        eng.dma_start(
            hall[:, half * 512:(half + 1) * 512].rearrange(
                "p (c i) -> p c i", i=128),
            hb_d.ap()[half * 512:(half + 1) * 512, :].rearrange(
                "(c p) i -> p c i", p=128))

    # ---- phase A: transpose h to hT [128 di, 1024 n] bf16; 2 f32r
    # transposes per PSUM bank, copies alternate between DVE and ACT so the
    # PSUM->SBUF drains pipeline with the next bank's transposes ----
    hT = consts.tile([128, N], BF16)
    for bank in range(4):
        pt = ps_small.tile([128, 256], F32R, tag="ps", padded_shape=[128, 512])
        for k in range(2):
            icn = bank * 2 + k
            nc.tensor.matmul(pt[:, k * 128:(k + 1) * 128],
                             hall[:, icn * 128:(icn + 1) * 128], ident[:],
                             is_transpose=True, start=(k == 0), stop=(k == 1),
                             skip_group_check=True)
        dst = hT[:, bank * 256:(bank + 1) * 256]
        if bank % 2 == 0:
            nc.vector.tensor_scalar(dst, pt[:], 0.0, None, ALU.add)
        else:
            nc.scalar.copy(dst, pt[:])

    # ---- phase C: st columns [128 n, jc*16 + (s_h | 8+t_h)]; 8 matmuls
    # packed into one PSUM bank; exp factors read straight from PSUM ----
    ps_c = ps_small.tile([128, 128], FP32, tag="ps", padded_shape=[128, 512])
    for jc in range(NC128):
        nc.tensor.matmul(ps_c[:, jc * 16:(jc + 1) * 16],
                         hT[:, jc * 128:(jc + 1) * 128], wq[:],
                         start=(jc == 0), stop=(jc == NC128 - 1),
                         skip_group_check=True)
    st3 = ps_c[:].rearrange("p (c q) -> p c q", q=16)
    # qcols = e^{0.8 s} as columns [128 n, (c h)] bf16
    qcols = consts.tile([128, NC128 * 8], DT_MM)
    nc.scalar.activation(qcols[:].rearrange("p (c q) -> p c q", q=8),
                         st3[:, :, 0:8], AF.Exp, scale=0.8)
    # f1 = e^t (moving-side scaling), rcols = e^{-0.8 t} (u-tile scalar)
    f1 = consts.tile([128, NC128 * 8], FP32)
    nc.scalar.activation(f1[:].rearrange("p (c q) -> p c q", q=8),
                         st3[:, :, 8:16], AF.Exp)
    rcols = consts.tile([128, NC128 * 8], FP32)
    nc.scalar.activation(rcols[:].rearrange("p (c q) -> p c q", q=8),
                         st3[:, :, 8:16], AF.Exp, scale=-0.8)

    # ---- warm the PE pstate while ACT works on the exps ----
    o4s_g = [ps_oh.tile([128, N], FP32, tag="oh", name=f"o4_{g}")
             for g in range(2)]

    def warm(n):
        for _ in range(n):
            nc.tensor.matmul(o4s_g[0][0:16, 0:512], wall[:, 0:16],
                             hT[:, 0:512], start=True, stop=True,
                             skip_group_check=True)

    warm(2)

    # ---- qT: transpose q columns to rows [8 h, 1024 n] in one PSUM bank,
    # then broadcast each row across partitions. Heads 0-1 use the GPSIMD
    # partition-broadcast (no DMA latency); the rest ride a DRAM bounce on
    # the SP/ACT DGE queues and arrive while earlier heads compute. ----
    qt = ps_small.tile([8, N], DT_MM, tag="ps", padded_shape=[128, N])
    for c in range(NC128):
        nc.tensor.matmul(qt[:, c * 128:(c + 1) * 128],
                         qcols[:, c * 8:(c + 1) * 8], identb[:],
                         is_transpose=True, start=(c == 0), stop=(c == NC128 - 1),
                         skip_group_check=True)
    qrows = consts.tile([8, N], DT_MM)
    nc.vector.tensor_scalar(qrows[:], qt[:], 0.0, None, ALU.add)
    qb = consts.tile([128, H * N], DT_MM)
    # head 0 via two GPSIMD partition-broadcast halves (SBUF only)
    nc.gpsimd.partition_broadcast(qb[:, 0:512], qrows[0:1, 0:512])
    nc.gpsimd.partition_broadcast(qb[:, 512:N], qrows[0:1, 512:N])

    # ---- phase D: g_ext [128 j, jc*256 + h*32 + d]; col 16 = ones (den),
    # cols 17..31 zero padding so matmuls cover all 128 psum partitions;
    # 4 matmuls packed per PSUM bank -> one strided copy per bank ----
    g_ext = consts.tile([128, NC128 * 256], DT_MM)
    g4 = g_ext[:].rearrange("p (c h q) -> p c h q", h=H, q=32)
    nc.vector.memset(g4[:, :, :, 16:17], 1.0)
    nc.vector.memset(g4[:, :, :, 17:32], 0.0)
    for half in range(2):
        pt = ps_small.tile([128, 512], FP32, tag="ps")
        for k in range(4):
            jc = half * 4 + k
            nc.tensor.matmul(pt[:, k * 128:(k + 1) * 128],
                             hT[:, jc * 128:(jc + 1) * 128], wall[:],
                             start=(k == 0), stop=(k == 3),
                             skip_group_check=True)
        dst = g_ext[:, half * 1024:(half + 1) * 1024].rearrange(
            "p (c h q) -> p c h q", h=H, q=32)[:, :, :, 0:16]
        src = pt[:].rearrange("p (c h q) -> p c h q", h=H, q=16)
        nc.scalar.copy(dst, src)

    # heads 1-7 ride a DRAM bounce; emitted after phase D so the g_ext
    # copies keep ACT-queue priority. The broadcasts must run on queues
    # OTHER than the writer's: same-queue DMA order is not a completion
    # order on HW, and only cross-queue readers get completion semaphores.
    qrows_d = dram.tile([H, N], DT_MM)
    nc.sync.dma_start(qrows_d[:], qrows[:])
    for h in range(1, H):
        eng = (nc.gpsimd, nc.scalar)[h % 2]
        eng.dma_start(qb[:, h * N:(h + 1) * N],
                      qrows_d[h:h + 1, :].partition_broadcast(128))

    warm(3)

    if variant == "ad":
        outt_all = consts.tile([128, NC128 * H * HD], FP32)
        nc.vector.memset(outt_all[:], 0.0)
        nc.sync.dma_start(
            out_d.ap().rearrange("(c p) i -> p c i", p=128),
            outt_all[:].rearrange("p (c i) -> p c i", i=H * HD))
        return

    # ---- phase E: u tiles (all on DVE) + attention matmuls (4 heads packed
    # per PSUM accumulator at partition offsets 32*hh via tile_position);
    # phase F work for group g is drip-fed into group g+1's jc loop ----
    outt_all = consts.tile([128, NC128 * H * HD], FP32)
    o4s = [consts.tile([128, N], DT_MM, name=f"o4s{g}") for g in range(2)]

    def emit_phase_f(grp, icn, dma_eng):
        t4 = ps_small.tile([128, 128], DT_MM, tag="ps", padded_shape=[128, N])
        nc.tensor.matmul(t4[:], o4s[grp][:, icn * 128:(icn + 1) * 128],
                         identb[:], is_transpose=True)
        t3 = t4[:].rearrange("p (a q) -> p a q", q=32)
        r4 = sb.tile([128, 4], FP32, tag="r4")
        nc.vector.reciprocal(r4[:], t3[:, :, 16:17])
        # relu was folded into the o4s copy; den > 0 so relu(num)/den is safe
        dst = outt_all[:].rearrange(
            "p (c h q) -> p c h q", h=H, q=HD)[:, icn, grp * 4:(grp + 1) * 4, :]
        nc.vector.tensor_tensor(dst, t3[:, :, 0:16], _bcast16(r4[:, 0:4]),
                                ALU.mult)
        if dma_eng is not None:
            # both groups' halves of i-chunk icn are done -> stream to HBM
            dma_eng.dma_start(
                out_d.ap()[icn * 128:(icn + 1) * 128, :],
                outt_all[:, icn * H * HD:(icn + 1) * H * HD])

    # head-major: head h's full jc accumulation only needs qb row h, so the
    # first matmul fires as soon as the first broadcast lands; later heads'
    # broadcasts stream in behind the ~3.4us/head PE pace
    for grp in range(2):
        o4 = o4s_g[grp]
        for hh in range(4):
            h = grp * 4 + hh
            last = hh == 3
            # the group's last head runs ih-outer so the ih=0 half of the
            # accumulator completes (and its copy + phase F fire) while the
            # ih=1 matmuls still stream
            mts = {}
            for jc in range(NC128):
                mt = mtp.tile([128, N], DT_MM, tag="mt", name=f"mt{hh}_{jc}")
                c = jc * 8 + h
                if h == 0 and jc == 0:
                    # head 0's first tile in halves: each GPSIMD broadcast
                    # half feeds its matmul without waiting for the other
                    for q in range(2):
                        nc.vector.tensor_scalar(
                            mt[:, q * 512:(q + 1) * 512],
                            qb[:, q * 512:(q + 1) * 512],
                            rcols[:, c:c + 1], f1[:, c:c + 1],
                            ALU.max, ALU.mult)
                else:
                    nc.vector.tensor_scalar(mt[:], qb[:, h * N:(h + 1) * N],
                                            rcols[:, c:c + 1],
                                            f1[:, c:c + 1],
                                            ALU.max, ALU.mult)
                mts[jc] = mt
                if not last:
                    _emit_mm(nc, o4, g_ext, mts, h, hh, (jc,),
                             ((0, 512), (512, 1024)))
            if last and grp == 1:
                # grp0's remaining phase F must precede the icn 6/7 HBM
                # writes that grp1's phase F below will issue
                emit_phase_f(0, 6, None)
                emit_phase_f(0, 7, None)
            if last:
                # column-staged accumulation for the group's last head: each
                # finished column group's copy + phase F + HBM write streams
                # out while the next group's matmuls still run. Phase F PE
                # transposes are emitted one matmul-group late so the PE
                # queue never stalls on the ACT copies.
                groups = ((0, 512), (512, 768), (768, 1024))
                pending = []
                for gi, (lo, hi) in enumerate(groups):
                    _emit_mm(nc, o4, g_ext, mts, h, hh, range(NC128),
                             ((lo, hi),))
                    if grp == 1 and pending:
                        for icn in pending:
                            emit_phase_f(1, icn,
                                         (nc.sync, nc.gpsimd)[icn % 2])
                        pending = []
                    # relu folded into the PSUM->SBUF copy (den > 0)
                    for q in range(lo // 256, hi // 256):
                        sl = slice(q * 256, (q + 1) * 256)
                        nc.scalar.activation(o4s[grp][:, sl], o4[:, sl],
                                             AF.Relu)
                    pending = list(range(lo // 128, hi // 128))
                if grp == 1:
                    for icn in pending:
                        emit_phase_f(1, icn, (nc.sync, nc.gpsimd)[icn % 2])
            if grp == 1 and hh < 3:
                # drip grp0's phase F into grp1's earlier head rounds
                emit_phase_f(0, hh * 2, None)
                emit_phase_f(0, hh * 2 + 1, None)


# ---- host wrapper ----
_CACHE = {}


def _prep_weights(W, Wa):
    W = np.asarray(W, dtype=np.float32)
    Wa = np.asarray(Wa, dtype=np.float32)
    wall = np.ascontiguousarray(W.transpose(1, 0, 2).reshape(DI, H * HD))
    wabd = np.zeros((DI, 2 * H), dtype=np.float32)
    for hh in range(H):
        wabd[hh * HD:(hh + 1) * HD, hh] = Wa[hh, :HD]
        wabd[hh * HD:(hh + 1) * HD, H + hh] = Wa[hh, HD:]
    wq = np.ascontiguousarray(wall @ wabd)   # s/t projections direct from hT
    ident = np.eye(128, dtype=np.float32)
    wall = wall.astype(ml_dtypes.bfloat16)
    wq = wq.astype(ml_dtypes.bfloat16)
    return wall, wq, ident


def kernel(h, W, Wa):
    h = np.asarray(h, dtype=np.float32)
    if "nc" not in _CACHE:
        _CACHE["nc"] = build_nc(iters=1)
    nc = _CACHE["nc"]
    wall, wabd, ident = _prep_weights(W, Wa)
    identb = ident.astype(ml_dtypes.bfloat16)
    in_maps = [
        {"hb": np.ascontiguousarray(h[c]), "wall": wall, "wabd": wabd,
         "ident": ident, "identb": identb}
        for c in range(B)
    ]
    res = bass_utils.run_bass_kernel_spmd(nc, in_maps, core_ids=list(range(B)))
    out = np.stack([res.results[c]["out"] for c in range(B)], axis=0)
    return out.astype(np.float32)
